# revision 65
# baseline (speedup 1.0000x reference)
"""CIN (Compressed Interaction Network) forward kernel for 8 Trainium2 NeuronCores.

Reference computation (per batch b, embedding dim d):
    x0 = inputs[b, :, d]                 # [F=39]
    h0 = x0
    for k in 0..2:
        z  = outer(x0, h_{k})            # [F * Hk]
        h_{k+1} = z @ Wk + bk            # [256]
    out[b] = concat_k sum_d h_{k+1}      # [768]

Strategy: data-parallel over batch (64 per core).  Per core, rows r = (b, d)
are 2048 GEMM rows.  Everything is laid out transposed: x0T[f, r], hT[u, r].

Layer 0 exploits z0 symmetry (x_i x_j = x_j x_i): only the 780 upper-triangle
pairs are kept, with W0 rows folded (W0[i,j] + W0[j,i] off-diagonal), so K
drops 1521 -> 780 (7 k-tiles instead of 13).  The pair products are built on
the host, scaled by 1/2 into fp8-e3m4 range (the x2 is folded into W0), and
streamed tile-major as the startup critical path on both HWDGE rings; the
DVE upconverts each tile to fp16 just ahead of the matmuls.  Total output
error from this quantization is ~1.3e-2 rel L2 (vs the 2e-2 gate).

Layer 1 is the full GEMM: z1[(i,j), r] = x0[i, r] * h1[j, r] built k-tile by
k-tile on the Vector engine from DMA-broadcast x0 rows.  It runs in TWO
column groups (batches 0-31, 32-63): group 0 finishes its whole K loop
first, so its h2 evacuation, transposes, and gram matmuls hook into group
1's matmul stream where they hide completely.

Layer 2's feature map is only ever used summed over d, so the full GEMM is
replaced by per-batch Grams: G2[b,i,j] = sum_d x0[b,i,d] h2[b,j,d], then
out2[u,b] = sum_{i,j} W2[(i,j),u] G2[b,i,j].  The u->d transpose of h2 runs
as full-width [128,512] DVE block-transposes (all four 32-row blocks per
pass) into a staging tile, and SBUF->SBUF gather DMAs (hidden on the HWDGE
rings) assemble the per-batch [d, u] layout the gram matmuls consume.  The
W2 contraction is h1-first so h0's gathers and grams hide under its first
half; filler matmuls bridge the transpose latency so the HAM clock governor
stays at full rate through the tail.

Known-fragile couplings (measured on HW, do not "simplify"):
- The staging tile h2dt is double-buffered per column group: gather DMA
  *source* reads are not WAR-tracked against later engine writes, so a
  single buffer races (wrong results, timing-dependent).
- A DMA whose tile pool has fewer buffers than outstanding tiles parks its
  dma_start in the issuing engine's FIFO, head-blocking every later
  instruction on that engine (z8p has bufs=7 for exactly this reason).
- DVE z-builds (~0.7us per [128,1024]) nearly saturate the Vector engine
  during layer 1, so the in-stream gram evacuations go to Scalar there,
  and to Vector (2.7x faster on strided writes) only in the tail.
"""

import os
import sys

import numpy as np

for _p in ("/opt/trn_rl_repo", "/root/.axon_site/_ro/trn_rl_repo"):
    if os.path.isdir(_p) and _p not in sys.path:
        sys.path.insert(0, _p)

N_CORES = 8
B, F, D = 512, 39, 32
U = 256
BL = B // N_CORES          # 64 batches per core
R = BL * D                 # 2048 GEMM rows per core
RG = R // 2                # layer-1 column group width (32 batches)
NB = 512                   # matmul moving free-dim (one PSUM bank of fp32)
NRB = R // NB              # 4 row blocks
NP = F * (F + 1) // 2      # 780 symmetric pairs for layer 0
KT0 = 7                    # layer-0 k-tiles: 6x128 + 1x12
KL0 = [128] * 6 + [NP - 768]
K12 = F * U                # 9984
KT12 = K12 // 128          # 78 k-tiles; kt = (i, half)
NWB = 8                    # gram matmuls (batches) per PSUM wave

DT = "float16"             # device compute dtype for z / W / h ("float16" | "bfloat16")

_prog_cache = {}


def _np_dt():
    import ml_dtypes

    return np.float16 if DT == "float16" else ml_dtypes.bfloat16


def _build_program():
    import concourse.mybir as mybir
    from concourse import bacc, tile

    dt = mybir.dt
    cdt = getattr(dt, DT)
    f32 = dt.float32

    nc = bacc.Bacc(
        "TRN2", target_bir_lowering=False, debug=False, num_devices=N_CORES
    )
    # tile-major (contiguous 256KB per k-tile), fp8-e3m4 with values
    # pre-scaled by 1/2 on the host (the x2 is folded into w0)
    z0_p = nc.declare_dram_parameter("z0", [KT0 * 128, R], dt.float8e3, isOutput=False)
    # x0 rows each replicated 32x in DRAM: broadcast DMAs read distinct
    # addresses (HBM bank spread) instead of hammering one 4KB row.
    x0r_p = nc.declare_dram_parameter("x0r", [F * 32, R], cdt, isOutput=False)
    x0d_p = nc.declare_dram_parameter("x0d", [32, BL, F], cdt, isOutput=False)
    w0_p = nc.declare_dram_parameter("w0", [128, KT0, U], cdt, isOutput=False)
    # chunk-major (contiguous 851KB per 13-ktile chunk)
    w1_p = nc.declare_dram_parameter("w1", [6 * 128, 13 * U], cdt, isOutput=False)
    w2_p = nc.declare_dram_parameter("w2", [6 * 128, 13 * U], cdt, isOutput=False)
    bias_p = nc.declare_dram_parameter("bias", [128, 4], f32, isOutput=False)
    out_p = nc.declare_dram_parameter("out", [128, 6, BL], f32, isOutput=True)

    with tile.TileContext(nc) as tc:
        with (
            tc.tile_pool(name="const", bufs=1) as constp,
            tc.tile_pool(name="wpool", bufs=1) as wpool,
            tc.tile_pool(name="xb", bufs=6) as xbp,
            tc.tile_pool(name="z8p", bufs=7) as z8p,
            tc.tile_pool(name="z0p", bufs=3) as z0p,
            tc.tile_pool(name="zp", bufs=3) as zp,
            tc.tile_pool(name="hp", bufs=1) as hp,
            tc.tile_pool(name="psum", bufs=1, space="PSUM") as psp,
        ):
            bcast_n = [0]

            def bcast(dst, src_ap):
                eng = nc.sync if bcast_n[0] % 2 == 0 else nc.scalar
                bcast_n[0] += 1
                eng.dma_start(dst, src_ap)

            out_sb = constp.tile([128, 6, BL], f32, tag="out")
            h_tiles = {
                (l, c): hp.tile([128, R], cdt, tag=f"h{l}{c}", name=f"h{l}{c}")
                for l in range(2)
                for c in range(2)
            }
            # layer-2 gram-path tiles: h2d[d, h, b, u_sub]
            h2d = hp.tile([32, 2, BL, 128], cdt, tag="h2d", name="h2d")
            # block-transpose staging: h2dt[g][(a,d), h, (b_local, du)] holds
            # one column group's in-place 32x32-transposed h2.  Per-group
            # buffers: the tail's writes must not race the in-group-1 gather
            # DMAs still reading group 0's staging.
            h2dt = [
                hp.tile([128, 2, RG], cdt, tag=f"h2dt{g}", name=f"h2dt{g}")
                for g in range(2)
            ]
            g2 = hp.tile([128, 2, F, BL], cdt, tag="g2", name="g2")
            x0d = constp.tile([32, BL, F], cdt, tag="x0d")

            w0 = wpool.tile([128, KT0, U], cdt, tag="w0")
            w1 = wpool.tile([128, KT12, U], cdt, tag="w1")
            w2 = wpool.tile([128, KT12, U], cdt, tag="w2")
            bias = constp.tile([128, 4], f32, tag="bias")

            def w_chunk(dst, src_p, c, eng):
                # chunk c of w1/w2: 13 k-tiles, contiguous in DRAM
                eng.dma_start(
                    dst[:, 13 * c : 13 * (c + 1), :],
                    src_p[c * 128 : (c + 1) * 128, :].rearrange(
                        "p (t u) -> p t u", u=U
                    ),
                )

            # ---- prologue: the z0 k-tiles are the startup critical path;
            # they go first on BOTH HWDGE rings (FIFO per ring) so layer 0
            # chases them at full aggregate line rate.  Only w0's first two
            # k-tiles (needed by the first layer-0 matmuls) jump ahead on
            # the scalar ring; everything layer 1 needs is queued behind z0
            # and lands during layer 0's ~12us of matmuls.
            z8_tiles = [
                z8p.tile([128, R], dt.float8e3, tag="z8", name=f"z8_{t}")
                for t in range(KT0)
            ]
            nc.scalar.dma_start(w0[:, :, :], w0_p[:, :, :])
            nc.scalar.dma_start(bias[:, :], bias_p[:, :])
            for t in range(KT0 - 1):
                eng = nc.sync if t % 2 == 0 else nc.scalar
                eng.dma_start(z8_tiles[t][:, :], z0_p[t * 128 : (t + 1) * 128, :])
            # last tile holds only 12 live pair rows; don't stream the pad
            nc.sync.dma_start(
                z8_tiles[6][: KL0[6], :], z0_p[6 * 128 : 6 * 128 + KL0[6], :]
            )

            # ---- PE warm-up: covers z0[0]+w0 DMA landing and spins the HAM
            # clock gate up (needs ~3.4us sustained matmul activity); runs
            # long enough that layer 0 starts with no PE-idle window.
            warm_ps = psp.tile([128, NB], f32, tag="ps_0_0", name="warm_ps")
            nc.vector.memset(h_tiles[(0, 0)][:, :NB], 0)
            for _ in range(17):
                nc.tensor.matmul(
                    warm_ps[:, :256],
                    h_tiles[(0, 0)][:, :128],
                    h_tiles[(0, 0)][:, :256],
                    start=True,
                    stop=True,
                )

            def make_x(g, i, nm, eng=None):
                t = xbp.tile([128, RG], cdt, tag="xi", name=nm)
                src = (
                    x0r_p[i * 32 : i * 32 + 32, g * RG : (g + 1) * RG]
                    .unsqueeze(1)
                    .to_broadcast((32, 4, RG))
                )
                if eng is None:
                    bcast(t[:, :], src)
                else:
                    eng.dma_start(t[:, :], src)
                return t

            # layer-1 group-0 head tiles + W1 chunk 0 ride the rings BEHIND
            # z0: they land while layer 0 is still matmul-ing.
            w_chunk(w1, w1_p, 0, nc.scalar)
            l1_pre = {(0, 0): make_x(0, 0, "l1xA0", eng=nc.sync)}
            l1_pre[(0, 1)] = make_x(0, 1, "l1xA1", eng=nc.scalar)
            l1_pre[(0, 2)] = make_x(0, 2, "l1xA2", eng=nc.sync)
            l1_pre[(0, 3)] = make_x(0, 3, "l1xA3", eng=nc.scalar)

            # ---- layer 0: symmetric-pair z streamed from DRAM as fp8,
            # upconverted to fp16 by the DVE one k-tile ahead, full R ----
            ps0 = [
                [
                    psp.tile([128, NB], f32, tag=f"ps_{c}_{r}", name=f"l0ps{c}{r}")
                    for r in range(NRB)
                ]
                for c in range(2)
            ]
            for kt in range(KT0):
                klen = KL0[kt]
                z0f = z0p.tile([128, R], cdt, tag="z0f", name=f"z0f{kt}")
                # fp8->fp16 upconvert runs at ~1 elem/lane/cycle; split each
                # tile across the Vector and Scalar engines to keep pace
                nc.vector.tensor_copy(
                    z0f[:klen, : R // 2], z8_tiles[kt][:klen, : R // 2]
                )
                nc.scalar.activation(
                    z0f[:klen, R // 2 :], z8_tiles[kt][:klen, R // 2 :],
                    mybir.ActivationFunctionType.Identity,
                )
                for c in range(2):
                    lhsT = w0[:klen, kt, c * 128 : (c + 1) * 128]
                    for r in range(NRB):
                        nc.tensor.matmul(
                            ps0[c][r][:, :],
                            lhsT,
                            z0f[:klen, r * NB : (r + 1) * NB],
                            start=(kt == 0),
                            stop=(kt == KT0 - 1),
                        )
            for r in range(NRB):
                for c in range(2):
                    if c == 0:
                        nc.vector.tensor_scalar_add(
                            h_tiles[(0, 0)][:, r * NB : (r + 1) * NB],
                            ps0[0][r][:, :],
                            bias[:, 0:1],
                        )
                    else:
                        nc.scalar.activation(
                            h_tiles[(0, 1)][:, r * NB : (r + 1) * NB],
                            ps0[1][r][:, :],
                            mybir.ActivationFunctionType.Identity,
                            bias=bias[:, 1:2],
                        )

            def h_reduce(l, col0=0, wdt=R):
                b0, nb = col0 // D, wdt // D
                for c in range(2):
                    nc.vector.tensor_reduce(
                        out_sb[:, l * 2 + c, b0 : b0 + nb],
                        h_tiles[(l, c)][:, col0 : col0 + wdt].rearrange(
                            "p (b d) -> p b d", d=D
                        ),
                        axis=mybir.AxisListType.X,
                        op=mybir.AluOpType.add,
                    )

            # ---- layer-2 building blocks (emitted via hooks) ----
            def emit_bst(h, g, half):
                # full-width in-place 32x32 block transpose of a [128, 512]
                # slab of h2 (all four 32-u-row blocks in one DVE pass):
                # h2dt[(a,d), (bl,du)] = h2[(a,du), (bl,d)]
                lo = g * RG + half * NB
                nc.vector.transpose(
                    h2dt[g][:, h, half * NB : (half + 1) * NB],
                    h_tiles[(1, h)][:, lo : lo + NB],
                )

            def emit_gather(h, a, g, eng):
                # SBUF->SBUF DMA: scatter staging rows into the per-batch
                # [d, u] gram layout (hidden on the HWDGE rings)
                eng.dma_start(
                    h2d[:, h, g * 32 : (g + 1) * 32, 32 * a : 32 * (a + 1)],
                    h2dt[g][32 * a : 32 * (a + 1), h, :].rearrange(
                        "d (b x) -> d b x", x=32
                    ),
                )

            wave_tags = ["ps_0_0", "ps_0_1", "ps_1_0", "ps_1_1"]
            wv_n = [0]

            def gram_wave(h, bg, filler=0, evac_eng=None):
                pt = psp.tile(
                    [128, NWB * F], f32,
                    tag=wave_tags[wv_n[0] % 4], name=f"gps{h}_{bg}",
                )
                wv_n[0] += 1
                # filler matmuls keep the HAM clock governor at full rate
                # through the exposed small-matmul tail; start=True on the
                # real grams below discards the garbage.
                for _ in range(filler):
                    nc.tensor.matmul(
                        pt[:, : NWB * F],
                        h_tiles[(0, 0)][:, :128],
                        h_tiles[(0, 0)][:, : NWB * F],
                        start=True,
                        stop=True,
                    )
                for g in range(NWB):
                    b = bg * NWB + g
                    nc.tensor.matmul(
                        pt[:, g * F : (g + 1) * F],
                        h2d[:, h, b, :],
                        x0d[:, b, :],
                        start=True,
                        stop=True,
                    )
                # psum wave -> G2 sbuf: scalar while the DVE is busy with
                # z-builds (group 1), vector in the tail where the DVE is
                # idle and ~2.7x faster on this strided transpose-write
                dst = g2[:, h, :, bg * NWB : (bg + 1) * NWB].rearrange(
                    "p i b -> p b i"
                )
                srcv = pt[:, :].rearrange("p (b i) -> p b i", i=F)
                if evac_eng is None:
                    nc.scalar.activation(
                        dst, srcv, mybir.ActivationFunctionType.Identity
                    )
                else:
                    # tail: split each evac across both engines so its wall
                    # time halves (it gates the W2 contraction start)
                    nc.vector.tensor_copy(dst[:, : NWB // 2], srcv[:, : NWB // 2])
                    nc.scalar.activation(
                        dst[:, NWB // 2 :], srcv[:, NWB // 2 :],
                        mybir.ActivationFunctionType.Identity,
                    )

            # ---- layer 1, one batch-column group ----
            def layer1_group(g, x_pre, z_pre, kt_hook, pre_evac=None):
                ps = {
                    (c, rr): psp.tile(
                        [128, NB], f32, tag=f"ps_{c}_{2 * g + rr}",
                        name=f"l1ps{g}_{c}{rr}",
                    )
                    for c in range(2)
                    for rr in range(2)
                }
                xcur = [None]
                for kt in range(KT12):
                    if kt_hook is not None:
                        kt_hook(kt)
                    i, half = kt // 2, kt % 2
                    if half == 0:
                        xcur[0] = (
                            x_pre[(g, i)] if (g, i) in x_pre
                            else make_x(g, i, f"x{g}_{i}")
                        )
                    if kt in z_pre:
                        z_t = z_pre[kt]
                    elif g == 0 and kt < 2:
                        # boundary pipelining vs layer-0 evacuation
                        z_t = zp.tile([128, RG], cdt, tag="z", name="zb")
                        for rr in range(2):
                            nc.vector.tensor_mul(
                                z_t[:, rr * NB : (rr + 1) * NB],
                                xcur[0][:, rr * NB : (rr + 1) * NB],
                                h_tiles[(0, half)][:, rr * NB : (rr + 1) * NB],
                            )
                    else:
                        z_t = zp.tile([128, RG], cdt, tag="z", name="zs")
                        nc.vector.tensor_mul(
                            z_t[:, :],
                            xcur[0][:, :],
                            h_tiles[(0, half)][:, g * RG : (g + 1) * RG],
                        )
                    for c in range(2):
                        lhsT = w1[:, kt, c * 128 : (c + 1) * 128]
                        for rr in range(2):
                            nc.tensor.matmul(
                                ps[(c, rr)][:, :],
                                lhsT,
                                z_t[:, rr * NB : (rr + 1) * NB],
                                start=(kt == 0),
                                stop=(kt == KT12 - 1),
                            )
                if pre_evac is not None:
                    pre_evac()
                for rr in range(2):
                    for c in range(2):
                        dst = h_tiles[(1, c)][
                            :, g * RG + rr * NB : g * RG + (rr + 1) * NB
                        ]
                        if c == 0:
                            nc.vector.tensor_scalar_add(
                                dst, ps[(0, rr)][:, :], bias[:, 2:3]
                            )
                        else:
                            nc.scalar.activation(
                                dst,
                                ps[(1, rr)][:, :],
                                mybir.ActivationFunctionType.Identity,
                                bias=bias[:, 3:4],
                            )

            # group 0: weight streaming + small-work hooks.  w2 chunks 0-2
            # load in late group 0, 3-5 in group 1; output DMAs ride the
            # gpsimd software DGE so their data dependencies never stall
            # the two hardware rings.
            w_sched = {0: 1, 3: 2, 8: 3, 13: 4, 20: 5}
            w2a_sched = {56: 0, 62: 1, 68: 2}

            def a_hook(kt):
                if kt == 4:
                    h_reduce(0)   # deferred layer-0 d-sum, off the boundary path
                if kt == 6:
                    nc.gpsimd.dma_start(out_p[:, 0:2, :], out_sb[:, 0:2, :])
                if kt == 30:
                    nc.sync.dma_start(x0d[:, :, :], x0d_p[:, :, :])
                if kt == 70:
                    l1_pre[(1, 0)] = make_x(1, 0, "l1xB0")
                if kt == 74:
                    l1_pre[(1, 1)] = make_x(1, 1, "l1xB1")
                if kt in w_sched:
                    c = w_sched[kt]
                    w_chunk(w1, w1_p, c, nc.sync if c % 2 else nc.scalar)
                if kt in w2a_sched:
                    c = w2a_sched[kt]
                    w_chunk(w2, w2_p, c, nc.sync if c % 2 else nc.scalar)

            # pre-build group-1's first two z tiles so the PE rolls straight
            # from group 0's last matmul into group 1 (the evacuations and
            # everything downstream then drain in group 1's shadow).
            zB_pre = {}

            def pre_evac_a():
                for kt in range(2):   # both kt 0, 1 use x0 row i=0
                    z_t = zp.tile([128, RG], cdt, tag="z", name=f"zB{kt}")
                    nc.vector.tensor_mul(
                        z_t[:, :],
                        l1_pre[(1, 0)][:, :],
                        h_tiles[(0, kt % 2)][:, RG:],
                    )
                    zB_pre[kt] = z_t

            layer1_group(0, l1_pre, {}, a_hook, pre_evac=pre_evac_a)

            # group 1: group-0's block transposes, gather DMAs, and gram
            # waves hook into this stream, spaced so the DVE's z-build
            # cadence absorbs them; h=1 first (the W2 contraction is
            # h1-first in the tail).
            bst_sched = {2: (1, 0), 5: (1, 1), 8: (0, 0), 11: (0, 1)}
            gat_sched = {8: (1, 0), 11: (1, 1), 14: (1, 2), 17: (1, 3),
                         20: (0, 0), 23: (0, 1), 26: (0, 2), 29: (0, 3)}
            wave_sched = {21: (1, 0), 25: (1, 1), 29: (1, 2), 33: (1, 3),
                          37: (0, 0), 41: (0, 1), 45: (0, 2), 49: (0, 3)}
            w2b_sched = {3: 3, 9: 4, 15: 5}

            def b_hook(kt):
                if kt in w2b_sched:
                    c = w2b_sched[kt]
                    w_chunk(w2, w2_p, c, nc.sync if c % 2 else nc.scalar)
                if kt in bst_sched:
                    h, half = bst_sched[kt]
                    emit_bst(h, 0, half)
                if kt in gat_sched:
                    h, a = gat_sched[kt]
                    emit_gather(h, a, 0, nc.sync if a % 2 else nc.scalar)
                if kt in wave_sched:
                    h, bg = wave_sched[kt]
                    gram_wave(h, bg)
                if kt == 54:
                    h_reduce(1, 0, RG)   # group-0 columns' d-sum, hidden here
                if kt == 58:
                    nc.gpsimd.dma_start(out_p[:, 2:4, 0:32], out_sb[:, 2:4, 0:32])

            layer1_group(1, l1_pre, zB_pre, b_hook)

            # ---- exposed tail: group 1's transposes/gathers/grams + the
            # W2 contraction.  Fillers bridge the transpose+gather latency
            # so the PE never idles long enough to re-throttle.
            ps_f = [
                psp.tile([128, BL], f32, tag="ps_0_2", name="psf0"),
                psp.tile([128, BL], f32, tag="ps_1_2", name="psf1"),
            ]

            def final_half(hi, h):
                for i in range(F):
                    for uh in range(2):
                        nc.tensor.matmul(
                            ps_f[uh][:, :],
                            w2[:, h * F + i, uh * 128 : (uh + 1) * 128],
                            g2[:, h, i, :],
                            start=(hi == 0 and i == 0),
                            stop=(hi == 1 and i == F - 1),
                        )

            for half in range(2):
                emit_bst(1, 1, half)
            for a in range(4):
                emit_gather(1, a, 1, nc.sync if a % 2 else nc.scalar)
            for half in range(2):
                emit_bst(0, 1, half)
            for a in range(4):
                emit_gather(0, a, 1, nc.sync if a % 2 else nc.scalar)
            # alternate evac engines: vector and scalar are equally slow on
            # this strided write (~1.8us), so run them pairwise in parallel
            gram_wave(1, 4, filler=24, evac_eng=nc.vector)
            gram_wave(1, 5, filler=6, evac_eng=nc.scalar)
            gram_wave(1, 6, evac_eng=nc.vector)
            gram_wave(1, 7, evac_eng=nc.scalar)
            for bg in range(4, 8):
                gram_wave(0, bg, evac_eng=nc.vector if bg % 2 == 0 else nc.scalar)
            final_half(0, 1)
            final_half(1, 0)

            h_reduce(1, RG, RG)   # group-1 columns' d-sum on DVE
            nc.gpsimd.dma_start(out_p[:, 2:4, 32:64], out_sb[:, 2:4, 32:64])
            for uh in range(2):
                nc.vector.tensor_copy(out_sb[:, 4 + uh, :], ps_f[uh][:, :])
            nc.sync.dma_start(out_p[:, 4:6, :], out_sb[:, 4:6, :])

    nc.compile()
    return nc


def _get_program():
    if "nc" not in _prog_cache:
        _prog_cache["nc"] = _build_program()
    return _prog_cache["nc"]


def _prep_maps(inputs):
    import ml_dtypes

    cdt = _np_dt()
    x = np.asarray(inputs["inputs"], np.float32)          # [512, 39, 32]
    Ws = [np.asarray(inputs[f"W{k}"], np.float32) for k in range(3)]
    bs = [np.asarray(inputs[f"b{k}"], np.float32) for k in range(3)]

    ii, jj = np.triu_indices(F)                           # 780 pairs, i-major

    # layer-0 weights: symmetric fold, pair p -> tile p//128, partition
    # p%128; the x2 compensates the host-side z0 fp8 prescale of 1/2
    w0r = Ws[0].reshape(F, F, U)
    w0s = np.where((ii == jj)[:, None], w0r[ii, jj], w0r[ii, jj] + w0r[jj, ii])
    w0t = np.zeros((KT0 * 128, U), np.float32)
    w0t[:NP] = 2.0 * w0s

    def chunked(w):
        # [128, KT12, U] -> chunk-major [6*128, 13*U]
        return (
            w.reshape(128, 6, 13, U).transpose(1, 0, 2, 3).reshape(6 * 128, 13 * U)
        )

    w_tiled = [
        w0t.reshape(KT0, 128, U).transpose(1, 0, 2).astype(cdt),
        chunked(Ws[1].reshape(KT12, 128, U).transpose(1, 0, 2)).astype(cdt),
        # W2 relayout for the gram contraction: [(i, j), u] ->
        # [j%128, (j//128)*F + i, u]
        chunked(
            Ws[2].reshape(F, 2, 128, U).transpose(2, 1, 0, 3).reshape(128, KT12, U)
        ).astype(cdt),
    ]
    w_tiled = [np.ascontiguousarray(w) for w in w_tiled]
    bias = np.zeros((128, 4), np.float32)
    for l in range(2):
        for c in range(2):
            bias[:, l * 2 + c] = bs[l][c * 128 : (c + 1) * 128]

    in_maps = []
    for core in range(N_CORES):
        xs = x[core * BL : (core + 1) * BL]               # [64, 39, 32]
        x0T = xs.transpose(1, 0, 2).reshape(F, R)         # fp32 [39, 2048]
        z0 = np.zeros((KT0 * 128, R), np.float32)
        z0[:NP] = 0.5 * (x0T[ii] * x0T[jj])               # fp8-e3m4 prescale
        z0t = np.ascontiguousarray(z0.astype(ml_dtypes.float8_e3m4))
        x0r = np.ascontiguousarray(np.repeat(x0T.astype(cdt), 32, axis=0))
        x0d = np.ascontiguousarray(xs.transpose(2, 0, 1).astype(cdt))
        in_maps.append(
            {
                "z0": z0t,
                "x0r": x0r,
                "x0d": x0d,
                "w0": w_tiled[0],
                "w1": w_tiled[1],
                "w2": w_tiled[2],
                "bias": bias,
            }
        )
    return in_maps, bs


def _finish_output(results, bs):
    outs = []
    for core in range(N_CORES):
        o = np.asarray(results[core]["out"], np.float32)  # [128, 6, 64]
        outs.append(o.transpose(2, 1, 0).reshape(BL, 768))
    out = np.concatenate(outs, axis=0)
    for l in range(3):
        out[:, l * U : (l + 1) * U] += D * bs[l]
    return np.ascontiguousarray(out.astype(np.float32))


def kernel(**inputs) -> np.ndarray:
    from concourse.bass_utils import run_bass_kernel_spmd

    in_maps, bs = _prep_maps(inputs)
    nc = _get_program()
    res = run_bass_kernel_spmd(nc, in_maps, list(range(N_CORES))).results
    return _finish_output(res, bs)


# revision 67
# speedup vs baseline: 1.0057x; 1.0057x over previous
"""CIN (Compressed Interaction Network) forward kernel for 8 Trainium2 NeuronCores.

Reference computation (per batch b, embedding dim d):
    x0 = inputs[b, :, d]                 # [F=39]
    h0 = x0
    for k in 0..2:
        z  = outer(x0, h_{k})            # [F * Hk]
        h_{k+1} = z @ Wk + bk            # [256]
    out[b] = concat_k sum_d h_{k+1}      # [768]

Strategy: data-parallel over batch (64 per core).  Per core, rows r = (b, d)
are 2048 GEMM rows.  Everything is laid out transposed: x0T[f, r], hT[u, r].

Layer 0 exploits z0 symmetry (x_i x_j = x_j x_i): only the 780 upper-triangle
pairs are kept, with W0 rows folded (W0[i,j] + W0[j,i] off-diagonal), so K
drops 1521 -> 780 (7 k-tiles instead of 13).  The pair products are built on
the host, scaled by 1/2 into fp8-e3m4 range (the x2 is folded into W0), and
streamed tile-major as the startup critical path on both HWDGE rings; the
DVE upconverts each tile to fp16 just ahead of the matmuls.  Total output
error from this quantization is ~1.3e-2 rel L2 (vs the 2e-2 gate).

Layer 1 is the full GEMM: z1[(i,j), r] = x0[i, r] * h1[j, r] built k-tile by
k-tile on the Vector engine from DMA-broadcast x0 rows.  It runs in TWO
column groups (batches 0-31, 32-63): group 0 finishes its whole K loop
first, so its h2 evacuation, transposes, and gram matmuls hook into group
1's matmul stream where they hide completely.

Layer 2's feature map is only ever used summed over d, so the full GEMM is
replaced by per-batch Grams: G2[b,i,j] = sum_d x0[b,i,d] h2[b,j,d], then
out2[u,b] = sum_{i,j} W2[(i,j),u] G2[b,i,j].  The u->d transpose of h2 runs
as full-width [128,512] DVE block-transposes (all four 32-row blocks per
pass) into a staging tile, and SBUF->SBUF gather DMAs (hidden on the HWDGE
rings) assemble the per-batch [d, u] layout the gram matmuls consume.  The
W2 contraction is h1-first so h0's gathers and grams hide under its first
half; filler matmuls bridge the transpose latency so the HAM clock governor
stays at full rate through the tail.

Known-fragile couplings (measured on HW, do not "simplify"):
- The staging tile h2dt is double-buffered per column group: gather DMA
  *source* reads are not WAR-tracked against later engine writes, so a
  single buffer races (wrong results, timing-dependent).
- A DMA whose tile pool has fewer buffers than outstanding tiles parks its
  dma_start in the issuing engine's FIFO, head-blocking every later
  instruction on that engine (z8p has bufs=7 for exactly this reason).
- DVE z-builds (~0.7us per [128,1024]) nearly saturate the Vector engine
  during layer 1, so the in-stream gram evacuations go to Scalar there,
  and to Vector (2.7x faster on strided writes) only in the tail.
"""

import os
import sys

import numpy as np

for _p in ("/opt/trn_rl_repo", "/root/.axon_site/_ro/trn_rl_repo"):
    if os.path.isdir(_p) and _p not in sys.path:
        sys.path.insert(0, _p)

N_CORES = 8
B, F, D = 512, 39, 32
U = 256
BL = B // N_CORES          # 64 batches per core
R = BL * D                 # 2048 GEMM rows per core
RG = R // 2                # layer-1 column group width (32 batches)
NB = 512                   # matmul moving free-dim (one PSUM bank of fp32)
NRB = R // NB              # 4 row blocks
NP = F * (F + 1) // 2      # 780 symmetric pairs for layer 0
KT0 = 7                    # layer-0 k-tiles: 6x128 + 1x12
KL0 = [128] * 6 + [NP - 768]
K12 = F * U                # 9984
KT12 = K12 // 128          # 78 k-tiles; kt = (i, half)
NWB = 8                    # gram matmuls (batches) per PSUM wave

DT = "float16"             # device compute dtype for z / W / h ("float16" | "bfloat16")

_prog_cache = {}


def _np_dt():
    import ml_dtypes

    return np.float16 if DT == "float16" else ml_dtypes.bfloat16


def _build_program():
    import concourse.mybir as mybir
    from concourse import bacc, tile

    dt = mybir.dt
    cdt = getattr(dt, DT)
    f32 = dt.float32

    nc = bacc.Bacc(
        "TRN2", target_bir_lowering=False, debug=False, num_devices=N_CORES
    )
    # tile-major (contiguous 256KB per k-tile), fp8-e3m4 with values
    # pre-scaled by 1/2 on the host (the x2 is folded into w0)
    z0_p = nc.declare_dram_parameter("z0", [KT0 * 128, R], dt.float8e3, isOutput=False)
    # x0 rows each replicated 32x in DRAM: broadcast DMAs read distinct
    # addresses (HBM bank spread) instead of hammering one 4KB row.
    x0r_p = nc.declare_dram_parameter("x0r", [F * 32, R], cdt, isOutput=False)
    x0d_p = nc.declare_dram_parameter("x0d", [32, BL, F], cdt, isOutput=False)
    w0_p = nc.declare_dram_parameter("w0", [128, KT0, U], cdt, isOutput=False)
    # chunk-major (contiguous 851KB per 13-ktile chunk)
    w1_p = nc.declare_dram_parameter("w1", [6 * 128, 13 * U], cdt, isOutput=False)
    w2_p = nc.declare_dram_parameter("w2", [6 * 128, 13 * U], cdt, isOutput=False)
    bias_p = nc.declare_dram_parameter("bias", [128, 4], f32, isOutput=False)
    out_p = nc.declare_dram_parameter("out", [128, 6, BL], f32, isOutput=True)

    with tile.TileContext(nc) as tc:
        with (
            tc.tile_pool(name="const", bufs=1) as constp,
            tc.tile_pool(name="wpool", bufs=1) as wpool,
            tc.tile_pool(name="xb", bufs=6) as xbp,
            tc.tile_pool(name="z8p", bufs=7) as z8p,
            tc.tile_pool(name="z0p", bufs=3) as z0p,
            tc.tile_pool(name="zp", bufs=3) as zp,
            tc.tile_pool(name="hp", bufs=1) as hp,
            tc.tile_pool(name="psum", bufs=1, space="PSUM") as psp,
        ):
            bcast_n = [0]

            def bcast(dst, src_ap):
                eng = nc.sync if bcast_n[0] % 2 == 0 else nc.scalar
                bcast_n[0] += 1
                eng.dma_start(dst, src_ap)

            out_sb = constp.tile([128, 6, BL], f32, tag="out")
            h_tiles = {
                (l, c): hp.tile([128, R], cdt, tag=f"h{l}{c}", name=f"h{l}{c}")
                for l in range(2)
                for c in range(2)
            }
            # layer-2 gram-path tiles: h2d[d, h, b, u_sub]
            h2d = hp.tile([32, 2, BL, 128], cdt, tag="h2d", name="h2d")
            # block-transpose staging: h2dt[g][(a,d), h, (b_local, du)] holds
            # one column group's in-place 32x32-transposed h2.  Per-group
            # buffers: the tail's writes must not race the in-group-1 gather
            # DMAs still reading group 0's staging.
            h2dt = [
                hp.tile([128, 2, RG], cdt, tag=f"h2dt{g}", name=f"h2dt{g}")
                for g in range(2)
            ]
            g2 = hp.tile([128, 2, F, BL], cdt, tag="g2", name="g2")
            x0d = constp.tile([32, BL, F], cdt, tag="x0d")

            w0 = wpool.tile([128, KT0, U], cdt, tag="w0")
            w1 = wpool.tile([128, KT12, U], cdt, tag="w1")
            w2 = wpool.tile([128, KT12, U], cdt, tag="w2")
            bias = constp.tile([128, 4], f32, tag="bias")

            def w_chunk(dst, src_p, c, eng):
                # chunk c of w1/w2: 13 k-tiles, contiguous in DRAM
                eng.dma_start(
                    dst[:, 13 * c : 13 * (c + 1), :],
                    src_p[c * 128 : (c + 1) * 128, :].rearrange(
                        "p (t u) -> p t u", u=U
                    ),
                )

            # ---- prologue: the z0 k-tiles are the startup critical path;
            # they go first on BOTH HWDGE rings (FIFO per ring) so layer 0
            # chases them at full aggregate line rate.  Only w0's first two
            # k-tiles (needed by the first layer-0 matmuls) jump ahead on
            # the scalar ring; everything layer 1 needs is queued behind z0
            # and lands during layer 0's ~12us of matmuls.
            z8_tiles = [
                z8p.tile([128, R], dt.float8e3, tag="z8", name=f"z8_{t}")
                for t in range(KT0)
            ]
            nc.scalar.dma_start(w0[:, :, :], w0_p[:, :, :])
            nc.scalar.dma_start(bias[:, :], bias_p[:, :])
            # each tile's column halves ride opposite rings, matching the
            # vector/scalar upconvert split: each upconvert half starts as
            # soon as its own half lands instead of waiting the full tile
            for t in range(KT0 - 1):
                nc.sync.dma_start(
                    z8_tiles[t][:, : R // 2],
                    z0_p[t * 128 : (t + 1) * 128, : R // 2],
                )
                nc.scalar.dma_start(
                    z8_tiles[t][:, R // 2 :],
                    z0_p[t * 128 : (t + 1) * 128, R // 2 :],
                )
            # last tile holds only 12 live pair rows; don't stream the pad
            nc.sync.dma_start(
                z8_tiles[6][: KL0[6], : R // 2],
                z0_p[6 * 128 : 6 * 128 + KL0[6], : R // 2],
            )
            nc.scalar.dma_start(
                z8_tiles[6][: KL0[6], R // 2 :],
                z0_p[6 * 128 : 6 * 128 + KL0[6], R // 2 :],
            )

            # ---- PE warm-up: covers z0[0]+w0 DMA landing and spins the HAM
            # clock gate up (needs ~3.4us sustained matmul activity); runs
            # long enough that layer 0 starts with no PE-idle window.
            warm_ps = psp.tile([128, NB], f32, tag="ps_0_0", name="warm_ps")
            nc.vector.memset(h_tiles[(0, 0)][:, :NB], 0)
            for _ in range(17):
                nc.tensor.matmul(
                    warm_ps[:, :256],
                    h_tiles[(0, 0)][:, :128],
                    h_tiles[(0, 0)][:, :256],
                    start=True,
                    stop=True,
                )

            def make_x(g, i, nm, eng=None):
                t = xbp.tile([128, RG], cdt, tag="xi", name=nm)
                src = (
                    x0r_p[i * 32 : i * 32 + 32, g * RG : (g + 1) * RG]
                    .unsqueeze(1)
                    .to_broadcast((32, 4, RG))
                )
                if eng is None:
                    bcast(t[:, :], src)
                else:
                    eng.dma_start(t[:, :], src)
                return t

            # layer-1 group-0 head tiles + W1 chunk 0 ride the rings BEHIND
            # z0: they land while layer 0 is still matmul-ing.
            w_chunk(w1, w1_p, 0, nc.scalar)
            l1_pre = {(0, 0): make_x(0, 0, "l1xA0", eng=nc.sync)}
            l1_pre[(0, 1)] = make_x(0, 1, "l1xA1", eng=nc.scalar)
            l1_pre[(0, 2)] = make_x(0, 2, "l1xA2", eng=nc.sync)
            l1_pre[(0, 3)] = make_x(0, 3, "l1xA3", eng=nc.scalar)

            # ---- layer 0: symmetric-pair z streamed from DRAM as fp8,
            # upconverted to fp16 by the DVE one k-tile ahead, full R ----
            ps0 = [
                [
                    psp.tile([128, NB], f32, tag=f"ps_{c}_{r}", name=f"l0ps{c}{r}")
                    for r in range(NRB)
                ]
                for c in range(2)
            ]
            for kt in range(KT0):
                klen = KL0[kt]
                z0f = z0p.tile([128, R], cdt, tag="z0f", name=f"z0f{kt}")
                # fp8->fp16 upconvert runs at ~1 elem/lane/cycle; split each
                # tile across the Vector and Scalar engines to keep pace
                nc.vector.tensor_copy(
                    z0f[:klen, : R // 2], z8_tiles[kt][:klen, : R // 2]
                )
                nc.scalar.activation(
                    z0f[:klen, R // 2 :], z8_tiles[kt][:klen, R // 2 :],
                    mybir.ActivationFunctionType.Identity,
                )
                for c in range(2):
                    lhsT = w0[:klen, kt, c * 128 : (c + 1) * 128]
                    for r in range(NRB):
                        nc.tensor.matmul(
                            ps0[c][r][:, :],
                            lhsT,
                            z0f[:klen, r * NB : (r + 1) * NB],
                            start=(kt == 0),
                            stop=(kt == KT0 - 1),
                        )
            for r in range(NRB):
                for c in range(2):
                    if c == 0:
                        nc.vector.tensor_scalar_add(
                            h_tiles[(0, 0)][:, r * NB : (r + 1) * NB],
                            ps0[0][r][:, :],
                            bias[:, 0:1],
                        )
                    else:
                        nc.scalar.activation(
                            h_tiles[(0, 1)][:, r * NB : (r + 1) * NB],
                            ps0[1][r][:, :],
                            mybir.ActivationFunctionType.Identity,
                            bias=bias[:, 1:2],
                        )

            def h_reduce(l, col0=0, wdt=R):
                b0, nb = col0 // D, wdt // D
                for c in range(2):
                    nc.vector.tensor_reduce(
                        out_sb[:, l * 2 + c, b0 : b0 + nb],
                        h_tiles[(l, c)][:, col0 : col0 + wdt].rearrange(
                            "p (b d) -> p b d", d=D
                        ),
                        axis=mybir.AxisListType.X,
                        op=mybir.AluOpType.add,
                    )

            # ---- layer-2 building blocks (emitted via hooks) ----
            def emit_bst(h, g, half):
                # full-width in-place 32x32 block transpose of a [128, 512]
                # slab of h2 (all four 32-u-row blocks in one DVE pass):
                # h2dt[(a,d), (bl,du)] = h2[(a,du), (bl,d)]
                lo = g * RG + half * NB
                nc.vector.transpose(
                    h2dt[g][:, h, half * NB : (half + 1) * NB],
                    h_tiles[(1, h)][:, lo : lo + NB],
                )

            def emit_gather(h, a, g, eng):
                # SBUF->SBUF DMA: scatter staging rows into the per-batch
                # [d, u] gram layout (hidden on the HWDGE rings)
                eng.dma_start(
                    h2d[:, h, g * 32 : (g + 1) * 32, 32 * a : 32 * (a + 1)],
                    h2dt[g][32 * a : 32 * (a + 1), h, :].rearrange(
                        "d (b x) -> d b x", x=32
                    ),
                )

            wave_tags = ["ps_0_0", "ps_0_1", "ps_1_0", "ps_1_1"]
            wv_n = [0]

            def gram_wave(h, bg, filler=0, evac_eng=None):
                pt = psp.tile(
                    [128, NWB * F], f32,
                    tag=wave_tags[wv_n[0] % 4], name=f"gps{h}_{bg}",
                )
                wv_n[0] += 1
                # filler matmuls keep the HAM clock governor at full rate
                # through the exposed small-matmul tail; start=True on the
                # real grams below discards the garbage.
                for _ in range(filler):
                    nc.tensor.matmul(
                        pt[:, : NWB * F],
                        h_tiles[(0, 0)][:, :128],
                        h_tiles[(0, 0)][:, : NWB * F],
                        start=True,
                        stop=True,
                    )
                for g in range(NWB):
                    b = bg * NWB + g
                    nc.tensor.matmul(
                        pt[:, g * F : (g + 1) * F],
                        h2d[:, h, b, :],
                        x0d[:, b, :],
                        start=True,
                        stop=True,
                    )
                # psum wave -> G2 sbuf: scalar while the DVE is busy with
                # z-builds (group 1), vector in the tail where the DVE is
                # idle and ~2.7x faster on this strided transpose-write
                dst = g2[:, h, :, bg * NWB : (bg + 1) * NWB].rearrange(
                    "p i b -> p b i"
                )
                srcv = pt[:, :].rearrange("p (b i) -> p b i", i=F)
                if evac_eng is None:
                    nc.scalar.activation(
                        dst, srcv, mybir.ActivationFunctionType.Identity
                    )
                else:
                    # tail: split each evac across both engines so its wall
                    # time halves (it gates the W2 contraction start)
                    nc.vector.tensor_copy(dst[:, : NWB // 2], srcv[:, : NWB // 2])
                    nc.scalar.activation(
                        dst[:, NWB // 2 :], srcv[:, NWB // 2 :],
                        mybir.ActivationFunctionType.Identity,
                    )

            # ---- layer 1, one batch-column group ----
            def layer1_group(g, x_pre, z_pre, kt_hook, pre_evac=None):
                ps = {
                    (c, rr): psp.tile(
                        [128, NB], f32, tag=f"ps_{c}_{2 * g + rr}",
                        name=f"l1ps{g}_{c}{rr}",
                    )
                    for c in range(2)
                    for rr in range(2)
                }
                xcur = [None]
                for kt in range(KT12):
                    if kt_hook is not None:
                        kt_hook(kt)
                    i, half = kt // 2, kt % 2
                    if half == 0:
                        xcur[0] = (
                            x_pre[(g, i)] if (g, i) in x_pre
                            else make_x(g, i, f"x{g}_{i}")
                        )
                    if kt in z_pre:
                        z_t = z_pre[kt]
                    elif g == 0 and kt < 2:
                        # boundary pipelining vs layer-0 evacuation
                        z_t = zp.tile([128, RG], cdt, tag="z", name="zb")
                        for rr in range(2):
                            nc.vector.tensor_mul(
                                z_t[:, rr * NB : (rr + 1) * NB],
                                xcur[0][:, rr * NB : (rr + 1) * NB],
                                h_tiles[(0, half)][:, rr * NB : (rr + 1) * NB],
                            )
                    else:
                        z_t = zp.tile([128, RG], cdt, tag="z", name="zs")
                        nc.vector.tensor_mul(
                            z_t[:, :],
                            xcur[0][:, :],
                            h_tiles[(0, half)][:, g * RG : (g + 1) * RG],
                        )
                    for c in range(2):
                        lhsT = w1[:, kt, c * 128 : (c + 1) * 128]
                        for rr in range(2):
                            nc.tensor.matmul(
                                ps[(c, rr)][:, :],
                                lhsT,
                                z_t[:, rr * NB : (rr + 1) * NB],
                                start=(kt == 0),
                                stop=(kt == KT12 - 1),
                            )
                if pre_evac is not None:
                    pre_evac()
                for rr in range(2):
                    for c in range(2):
                        dst = h_tiles[(1, c)][
                            :, g * RG + rr * NB : g * RG + (rr + 1) * NB
                        ]
                        if c == 0:
                            nc.vector.tensor_scalar_add(
                                dst, ps[(0, rr)][:, :], bias[:, 2:3]
                            )
                        else:
                            nc.scalar.activation(
                                dst,
                                ps[(1, rr)][:, :],
                                mybir.ActivationFunctionType.Identity,
                                bias=bias[:, 3:4],
                            )

            # group 0: weight streaming + small-work hooks.  w2 chunks 0-2
            # load in late group 0, 3-5 in group 1; output DMAs ride the
            # gpsimd software DGE so their data dependencies never stall
            # the two hardware rings.
            w_sched = {0: 1, 3: 2, 8: 3, 13: 4, 20: 5}
            w2a_sched = {56: 0, 62: 1, 68: 2}

            def a_hook(kt):
                if kt == 4:
                    h_reduce(0)   # deferred layer-0 d-sum, off the boundary path
                if kt == 6:
                    nc.gpsimd.dma_start(out_p[:, 0:2, :], out_sb[:, 0:2, :])
                if kt == 30:
                    nc.sync.dma_start(x0d[:, :, :], x0d_p[:, :, :])
                if kt == 70:
                    l1_pre[(1, 0)] = make_x(1, 0, "l1xB0")
                if kt == 74:
                    l1_pre[(1, 1)] = make_x(1, 1, "l1xB1")
                if kt in w_sched:
                    c = w_sched[kt]
                    w_chunk(w1, w1_p, c, nc.sync if c % 2 else nc.scalar)
                if kt in w2a_sched:
                    c = w2a_sched[kt]
                    w_chunk(w2, w2_p, c, nc.sync if c % 2 else nc.scalar)

            # pre-build group-1's first two z tiles so the PE rolls straight
            # from group 0's last matmul into group 1 (the evacuations and
            # everything downstream then drain in group 1's shadow).
            zB_pre = {}

            def pre_evac_a():
                for kt in range(2):   # both kt 0, 1 use x0 row i=0
                    z_t = zp.tile([128, RG], cdt, tag="z", name=f"zB{kt}")
                    nc.vector.tensor_mul(
                        z_t[:, :],
                        l1_pre[(1, 0)][:, :],
                        h_tiles[(0, kt % 2)][:, RG:],
                    )
                    zB_pre[kt] = z_t

            layer1_group(0, l1_pre, {}, a_hook, pre_evac=pre_evac_a)

            # group 1: group-0's block transposes, gather DMAs, and gram
            # waves hook into this stream, spaced so the DVE's z-build
            # cadence absorbs them; h=1 first (the W2 contraction is
            # h1-first in the tail).
            bst_sched = {2: (1, 0), 5: (1, 1), 8: (0, 0), 11: (0, 1)}
            gat_sched = {8: (1, 0), 11: (1, 1), 14: (1, 2), 17: (1, 3),
                         20: (0, 0), 23: (0, 1), 26: (0, 2), 29: (0, 3)}
            wave_sched = {21: (1, 0), 25: (1, 1), 29: (1, 2), 33: (1, 3),
                          37: (0, 0), 41: (0, 1), 45: (0, 2), 49: (0, 3)}
            w2b_sched = {3: 3, 9: 4, 15: 5}

            def b_hook(kt):
                if kt in w2b_sched:
                    c = w2b_sched[kt]
                    w_chunk(w2, w2_p, c, nc.sync if c % 2 else nc.scalar)
                if kt in bst_sched:
                    h, half = bst_sched[kt]
                    emit_bst(h, 0, half)
                if kt in gat_sched:
                    h, a = gat_sched[kt]
                    emit_gather(h, a, 0, nc.sync if a % 2 else nc.scalar)
                if kt in wave_sched:
                    h, bg = wave_sched[kt]
                    gram_wave(h, bg)
                if kt == 54:
                    h_reduce(1, 0, RG)   # group-0 columns' d-sum, hidden here
                if kt == 58:
                    nc.gpsimd.dma_start(out_p[:, 2:4, 0:32], out_sb[:, 2:4, 0:32])

            layer1_group(1, l1_pre, zB_pre, b_hook)

            # ---- exposed tail: group 1's transposes/gathers/grams + the
            # W2 contraction.  Fillers bridge the transpose+gather latency
            # so the PE never idles long enough to re-throttle.
            ps_f = [
                psp.tile([128, BL], f32, tag="ps_0_2", name="psf0"),
                psp.tile([128, BL], f32, tag="ps_1_2", name="psf1"),
            ]

            def final_half(hi, h):
                for i in range(F):
                    for uh in range(2):
                        nc.tensor.matmul(
                            ps_f[uh][:, :],
                            w2[:, h * F + i, uh * 128 : (uh + 1) * 128],
                            g2[:, h, i, :],
                            start=(hi == 0 and i == 0),
                            stop=(hi == 1 and i == F - 1),
                        )

            for half in range(2):
                emit_bst(1, 1, half)
            for a in range(4):
                emit_gather(1, a, 1, nc.sync if a % 2 else nc.scalar)
            for half in range(2):
                emit_bst(0, 1, half)
            for a in range(4):
                emit_gather(0, a, 1, nc.sync if a % 2 else nc.scalar)
            # alternate evac engines: vector and scalar are equally slow on
            # this strided write (~1.8us), so run them pairwise in parallel
            gram_wave(1, 4, filler=10, evac_eng=nc.vector)
            gram_wave(1, 5, filler=4, evac_eng=nc.scalar)
            gram_wave(1, 6, evac_eng=nc.vector)
            gram_wave(1, 7, evac_eng=nc.scalar)
            for bg in range(4, 8):
                gram_wave(0, bg, evac_eng=nc.vector if bg % 2 == 0 else nc.scalar)
            final_half(0, 1)
            final_half(1, 0)

            h_reduce(1, RG, RG)   # group-1 columns' d-sum on DVE
            nc.gpsimd.dma_start(out_p[:, 2:4, 32:64], out_sb[:, 2:4, 32:64])
            for uh in range(2):
                nc.vector.tensor_copy(out_sb[:, 4 + uh, :], ps_f[uh][:, :])
            nc.sync.dma_start(out_p[:, 4:6, :], out_sb[:, 4:6, :])

    nc.compile()
    return nc


def _get_program():
    if "nc" not in _prog_cache:
        _prog_cache["nc"] = _build_program()
    return _prog_cache["nc"]


def _prep_maps(inputs):
    import ml_dtypes

    cdt = _np_dt()
    x = np.asarray(inputs["inputs"], np.float32)          # [512, 39, 32]
    Ws = [np.asarray(inputs[f"W{k}"], np.float32) for k in range(3)]
    bs = [np.asarray(inputs[f"b{k}"], np.float32) for k in range(3)]

    ii, jj = np.triu_indices(F)                           # 780 pairs, i-major

    # layer-0 weights: symmetric fold, pair p -> tile p//128, partition
    # p%128; the x2 compensates the host-side z0 fp8 prescale of 1/2
    w0r = Ws[0].reshape(F, F, U)
    w0s = np.where((ii == jj)[:, None], w0r[ii, jj], w0r[ii, jj] + w0r[jj, ii])
    w0t = np.zeros((KT0 * 128, U), np.float32)
    w0t[:NP] = 2.0 * w0s

    def chunked(w):
        # [128, KT12, U] -> chunk-major [6*128, 13*U]
        return (
            w.reshape(128, 6, 13, U).transpose(1, 0, 2, 3).reshape(6 * 128, 13 * U)
        )

    w_tiled = [
        w0t.reshape(KT0, 128, U).transpose(1, 0, 2).astype(cdt),
        chunked(Ws[1].reshape(KT12, 128, U).transpose(1, 0, 2)).astype(cdt),
        # W2 relayout for the gram contraction: [(i, j), u] ->
        # [j%128, (j//128)*F + i, u]
        chunked(
            Ws[2].reshape(F, 2, 128, U).transpose(2, 1, 0, 3).reshape(128, KT12, U)
        ).astype(cdt),
    ]
    w_tiled = [np.ascontiguousarray(w) for w in w_tiled]
    bias = np.zeros((128, 4), np.float32)
    for l in range(2):
        for c in range(2):
            bias[:, l * 2 + c] = bs[l][c * 128 : (c + 1) * 128]

    in_maps = []
    for core in range(N_CORES):
        xs = x[core * BL : (core + 1) * BL]               # [64, 39, 32]
        x0T = xs.transpose(1, 0, 2).reshape(F, R)         # fp32 [39, 2048]
        z0 = np.zeros((KT0 * 128, R), np.float32)
        z0[:NP] = 0.5 * (x0T[ii] * x0T[jj])               # fp8-e3m4 prescale
        z0t = np.ascontiguousarray(z0.astype(ml_dtypes.float8_e3m4))
        x0r = np.ascontiguousarray(np.repeat(x0T.astype(cdt), 32, axis=0))
        x0d = np.ascontiguousarray(xs.transpose(2, 0, 1).astype(cdt))
        in_maps.append(
            {
                "z0": z0t,
                "x0r": x0r,
                "x0d": x0d,
                "w0": w_tiled[0],
                "w1": w_tiled[1],
                "w2": w_tiled[2],
                "bias": bias,
            }
        )
    return in_maps, bs


def _finish_output(results, bs):
    outs = []
    for core in range(N_CORES):
        o = np.asarray(results[core]["out"], np.float32)  # [128, 6, 64]
        outs.append(o.transpose(2, 1, 0).reshape(BL, 768))
    out = np.concatenate(outs, axis=0)
    for l in range(3):
        out[:, l * U : (l + 1) * U] += D * bs[l]
    return np.ascontiguousarray(out.astype(np.float32))


def kernel(**inputs) -> np.ndarray:
    from concourse.bass_utils import run_bass_kernel_spmd

    in_maps, bs = _prep_maps(inputs)
    nc = _get_program()
    res = run_bass_kernel_spmd(nc, in_maps, list(range(N_CORES))).results
    return _finish_output(res, bs)
